# revision 36
# baseline (speedup 1.0000x reference)
"""Trainium2 Bass kernel for the DialogGCN GAT-style message-passing layer.

Math notes (why this is much cheaper than the reference graph):
  Kp    = concat(K, kfeat) @ Wk + bk                    (B,N,D)
  alpha = Q@wden[:D] + Kp@wden[D:] + bden               (B,N)
  w     = softmax(alpha - (1-adj)*1e30, axis=N)
  out   = sum_n w * ((Kp@Wr0)*sm + (Kp@Wr1)*(1-sm))

* softmax is invariant to per-row constants, so the Q term, bden and the
  bk@wden[D:] constant all cancel:  w = softmax_n(X_n . v) masked, where
  X = concat(K, kfeat) and v = Wk @ wden[D:]  (folded on host).
* the output is linear in the weighted sums:
    out = (sum w*sm*[X|1]) @ [Wk;bk] @ Wr0 + (sum w*(1-sm)*[X|1]) @ [Wk;bk] @ Wr1
  so G0 = [Wk;bk]@Wr0 and G1 = [Wk;bk]@Wr1 are folded on host (769x512 each)
  and the device only needs one streaming pass over X computing
    s_n = X_n . v ; p_n = exp(s_n)*adj_n ; U0/U1 = sum pp_n*[X|1]
  followed by a tiny projection (U0@G0 + U1@G1) / P.
* masked tokens (adj=0) contribute exactly zero (the reference's -1e30 shift
  underflows exp to 0.0), so the host packs each (core, branch)'s valid
  tokens across all 4 batch rows into one token list padded to NTB*128
  (~4224 vs 4*2048).  Batch membership and the speaker mask are carried by
  a host-built 8-column mask m8 (col b = sm, col 4+b = 1-sm for batch-b
  tokens).  A host-appended ones column (x[:,768]=1) makes the U matmul
  also produce the softmax denominators P0/P1 for free.
* streams/weights ship as fp16 (rel-err budget 2e-2; fp16 adds ~1e-3).

Device pipeline per token group (~8 tiles of 128 tokens):
  DMA x [128, g, 770] fp16  (SP HWDGE queue)
  scores: one DVE fp16 2x multiply + one Pool multiply -> prod = x*v;
  row sums: one segmented DVE tensor_reduce (n<rn) + per-n ACT
  copy-accumulates (n>=rn); ACT exp; pp = p * m8 (one broadcast multiply).
  PE: per tile 2 fp16 matmuls accumulate U in PSUM ([8,512] + [8,258]).
  Finish per branch: U -> transpose -> (U0@G0 + U1@G1)/P -> out DMA.

Sharding: pure data parallel over batch B=32 across 8 cores (4 rows each).
"""

import os
import sys

import numpy as np

for _p in ("/opt/trn_rl_repo", "/root/.axon_site/_ro/trn_rl_repo"):
    if os.path.isdir(_p) and _p not in sys.path:
        sys.path.insert(0, _p)

B, N, D, KD = 32, 2048, 512, 256
F = D + KD  # 768
FP = F + 2  # 770: ones column (768) + pad (769)
NCORES = 8
BL = B // NCORES  # 4 batch rows per core

_BUILD_CACHE = {}
last_results = None  # BassKernelResults of the most recent run (for test.py)


def _groups(NTB: int, small_first: bool):
    """Split NTB tiles into ~4 groups; a small edge group shortens the
    pipeline fill (front branch) / drain (back branch)."""
    if NTB <= 6:
        return [(0, NTB)]
    edge = min(6, NTB - 1)
    rest = NTB - edge
    ng = max(1, round(rest / 9))
    bounds = [round(rest * i / ng) for i in range(ng + 1)]
    if small_first:
        return [(0, edge)] + [(edge + a, edge + b) for a, b in zip(bounds, bounds[1:])]
    return [(a, b) for a, b in zip(bounds, bounds[1:])] + [(rest, NTB)]


def _build(NTB: int, ps: int, rn: int):
    """Trace the Bass program (same NEFF runs SPMD on all 8 cores).

    NTB : packed token tiles per branch (context = 128*NTB tokens)
    ps  : score multiply columns done by the Pool engine ([F-ps:F))
    rn  : per-group score tiles row-summed by DVE tensor_reduce; the ACT
          engine covers the rest with full-row copy-accumulates
    """
    import concourse.bass as bass
    import concourse.tile as tile
    from concourse import bacc, mybir
    from concourse.masks import make_identity

    f32 = mybir.dt.float32
    f16 = mybir.dt.float16
    CB = 128 * NTB
    MS = F - ps  # DVE multiply slice [0:MS)

    nc = bacc.Bacc()

    ins = {}
    for s in ("f", "b"):
        ins[f"x_{s}"] = nc.dram_tensor(f"x_{s}", [CB, FP], f16, kind="ExternalInput")
        ins[f"m8_{s}"] = nc.dram_tensor(f"m8_{s}", [128, NTB, 8], f16, kind="ExternalInput")
        ins[f"v_{s}"] = nc.dram_tensor(f"v_{s}", [F], f16, kind="ExternalInput")
        ins[f"G0_{s}"] = nc.dram_tensor(f"G0_{s}", [F + 1, D], f16, kind="ExternalInput")
        ins[f"G1_{s}"] = nc.dram_tensor(f"G1_{s}", [F + 1, D], f16, kind="ExternalInput")
        ins[f"out_{s}"] = nc.dram_tensor(f"out_{s}", [BL, D], f32, kind="ExternalOutput")

    with tile.TileContext(nc) as tc:
        with (
            tc.tile_pool(name="singles", bufs=1) as singles,
            tc.tile_pool(name="xp", bufs=6) as xp,
            tc.tile_pool(name="scr", bufs=3) as scr,
            tc.tile_pool(name="small", bufs=4) as small,
            tc.tile_pool(name="ppp", bufs=3) as ppp,
            tc.tile_pool(name="uallp", bufs=2) as uallp,
            tc.tile_pool(name="uallTp", bufs=2) as uallTp,
            tc.tile_pool(name="finp", bufs=2) as finp,
            tc.tile_pool(name="psU_K", bufs=1, space="PSUM") as psU_K,
            tc.tile_pool(name="psU_1", bufs=1, space="PSUM") as psU_1,
            tc.tile_pool(name="psPp", bufs=1, space="PSUM") as psPp,
            tc.tile_pool(name="psTr", bufs=2, space="PSUM") as psTr,
            tc.tile_pool(name="psOut", bufs=2, space="PSUM") as psOut,
        ):
            # ---- one-time setup -------------------------------------------
            identh = singles.tile([128, 128], f16)
            make_identity(nc, identh)
            ones11h = singles.tile([1, 1], f16)
            nc.gpsimd.memset(ones11h, 1.0)

            st = {}
            for s in ("f", "b"):
                d = {}
                vb = singles.tile([128, F], f16, tag=f"vb_{s}")
                vap = ins[f"v_{s}"][:]
                nc.scalar.dma_start(
                    out=vb,
                    in_=bass.AP(tensor=vap.tensor, offset=vap.offset, ap=[[0, 128]] + vap.ap),
                )
                d["vb"] = vb
                m8s = singles.tile([128, NTB, 8], f16, tag=f"m8_{s}")
                nc.scalar.dma_start(out=m8s, in_=ins[f"m8_{s}"][:, :, :])
                d["m8"] = m8s
                st[s] = d

            def load_G(s, which):
                # G matrices: (128, 7, 512); chunk 6 row 0 holds row 768.
                # Issued mid-pipeline (ACT queue) so the 3.2MB of weights
                # doesn't compete with the first token streams for DMA.
                g = ins[f"G{which}_{s}"]
                gs = singles.tile([128, 7, D], f16, tag=f"G{which}_{s}")
                nc.scalar.dma_start(
                    out=gs[:, 0:6, :],
                    in_=g[0:F, :].rearrange("(k p) n -> p k n", p=128),
                )
                nc.scalar.dma_start(out=gs[0:1, 6, :], in_=g[F : F + 1, :])
                st[s][f"G{which}"] = gs

            def bcast_mid(ap2d, lo, hi, cnt):
                # [128, K] slice -> [128, cnt, K] with a stride-0 middle dim
                sl = ap2d[:, lo:hi]
                return bass.AP(
                    tensor=sl.tensor, offset=sl.offset, ap=[sl.ap[0], [0, cnt], sl.ap[1]]
                )

            def bcast_last(ap2d, lo, hi, cnt):
                # [128, K] slice -> [128, K, cnt] with a stride-0 last dim
                sl = ap2d[:, lo:hi]
                return bass.AP(
                    tensor=sl.tensor, offset=sl.offset, ap=[sl.ap[0], sl.ap[1], [0, cnt]]
                )

            # ---- streaming + finishing per branch -------------------------
            gmax = max(
                hi - lo for sf in (True, False) for lo, hi in _groups(NTB, sf)
            )
            for si, s in enumerate(("f", "b")):
                d = st[s]
                psK = psU_K.tile([8, D], f32)  # rows 0-3: U0(b), rows 4-7: U1(b)
                ps1 = psU_1.tile([8, KD + 2], f32)  # col KD holds P0/P1
                groups = _groups(NTB, small_first=(si == 0))
                xsrc = ins[f"x_{s}"].rearrange("(p n) d -> p n d", n=NTB)

                for gi, (lo, hi) in enumerate(groups):
                    g = hi - lo
                    first_g = si == 0 and gi == 0
                    last_g = si == 1 and gi == len(groups) - 1
                    chunks = (
                        [(a, min(a + 3, g)) for a in range(0, g, 3)]
                        if (first_g or last_g)
                        else [(0, g)]
                    )

                    x = xp.tile([128, gmax, FP], f16, tag="x")
                    # first group goes through gpsimd SWDGE: descriptor
                    # generation is ~0.34ns/desc vs ~45ns/desc on the SP
                    # HWDGE, so the very first tile lands ~5us earlier
                    xq = nc.gpsimd if first_g else nc.sync
                    for c0, c1 in chunks:
                        xq.dma_start(
                            out=x[:, c0:c1, :], in_=xsrc[:, lo + c0 : lo + c1, :]
                        )

                    prodS = scr.tile([128, gmax, F], f16, tag="prodS")
                    junkS = scr.tile([128, F], f16, tag="junkS")
                    sS = small.tile([128, g], f32, tag="sS")
                    for c0, c1 in chunks:
                        # elementwise x*v products; DVE runs fp16 in 2x mode
                        nc.vector.tensor_mul(
                            prodS[:, c0:c1, 0:MS],
                            x[:, c0:c1, 0:MS],
                            bcast_mid(d["vb"], 0, MS, c1 - c0),
                        )
                        if ps:
                            nc.gpsimd.tensor_mul(
                                prodS[:, c0:c1, MS:F],
                                x[:, c0:c1, MS:F],
                                bcast_mid(d["vb"], MS, F, c1 - c0),
                            )
                    # per-token sums: DVE does n<rn in one segmented reduce,
                    # ACT accumulates full rows for n>=rn
                    rg = min(rn, g)
                    nc.vector.tensor_reduce(
                        out=sS[:, 0:rg],
                        in_=prodS[:, 0:rg, :],
                        axis=mybir.AxisListType.X,
                        op=mybir.AluOpType.add,
                    )
                    for n in range(rg, g):
                        nc.scalar.activation(
                            out=junkS,
                            in_=prodS[:, n, :],
                            func=mybir.ActivationFunctionType.Copy,
                            accum_out=sS[:, n : n + 1],
                        )
                    p_raw = small.tile([128, g], f32, tag="p_raw")
                    nc.scalar.activation(
                        out=p_raw, in_=sS, func=mybir.ActivationFunctionType.Exp
                    )

                    # pp[:, n, c]: p * m8 selects batch column + speaker group
                    pp = ppp.tile([128, g, 8], f16, tag="pp")
                    nc.vector.tensor_mul(
                        pp, d["m8"][:, lo:hi, :], bcast_last(p_raw, 0, g, 8)
                    )

                    for n in range(g):
                        first = gi == 0 and n == 0
                        last = gi == len(groups) - 1 and n == g - 1
                        nc.tensor.matmul(
                            psK, pp[:, n, :], x[:, n, 0:D], start=first, stop=last
                        )
                        nc.tensor.matmul(
                            ps1, pp[:, n, :], x[:, n, D:FP], start=first, stop=last
                        )

                    # stagger the weight loads out of the contended early
                    # DMA window: G_f mid-branch-f, G_b once branch b's
                    # streams start draining the queue
                    if s == "f" and gi == 2:
                        load_G("f", 0)
                        load_G("f", 1)
                    if s == "b" and gi == 1:
                        load_G("b", 0)
                        load_G("b", 1)

                # ---- finishing: out = (U0@G0 + U1@G1) / P ------------------
                uall = uallp.tile([8, F + 1], f16)
                nc.vector.tensor_copy(uall[:, 0:D], psK)
                nc.vector.tensor_copy(uall[:, D : F + 1], ps1[:, 0 : KD + 1])

                uallT = uallTp.tile([128, 7, 8], f16)
                for k in range(6):
                    trp = psTr.tile([128, 8], f16)
                    nc.tensor.transpose(trp, uall[:, k * 128 : (k + 1) * 128], identh[0:8, 0:8])
                    nc.vector.tensor_copy(uallT[:, k, :], trp)
                trp = psTr.tile([128, 8], f16)
                nc.tensor.transpose(trp[0:1, :], uall[:, F : F + 1], identh[0:8, 0:8])
                nc.vector.tensor_copy(uallT[0:1, 6, :], trp[0:1, :])

                po = psOut.tile([4, D], f32)
                for k in range(6):
                    nc.tensor.matmul(
                        po, uallT[:, k, 0:4], d["G0"][:, k, :], start=(k == 0), stop=False
                    )
                nc.tensor.matmul(po, uallT[0:1, 6, 0:4], d["G0"][0:1, 6, :], start=False, stop=False)
                for k in range(6):
                    nc.tensor.matmul(po, uallT[:, k, 4:8], d["G1"][:, k, :], start=False, stop=False)
                nc.tensor.matmul(po, uallT[0:1, 6, 4:8], d["G1"][0:1, 6, :], start=False, stop=True)

                psP4 = psPp.tile([4, 1], f32, tag="psP")
                nc.tensor.matmul(psP4, uallT[0:1, 6, 0:4], ones11h, start=True, stop=False)
                nc.tensor.matmul(psP4, uallT[0:1, 6, 4:8], ones11h, start=False, stop=True)

                rp = finp.tile([4, 1], f32, tag="rp")
                nc.vector.reciprocal(rp, psP4)
                osb = finp.tile([4, D], f32, tag="osb")
                nc.vector.tensor_scalar_mul(out=osb, in0=po, scalar1=rp)
                d["osb"] = osb

            # outputs issued at program end on the SP queue: by then all
            # stream dma_starts are already enqueued, so the dep waits here
            # cannot stall anything
            for s in ("f", "b"):
                nc.sync.dma_start(out=ins[f"out_{s}"][:, :], in_=st[s]["osb"])

    nc.compile()
    return nc


def _get_nc(NTB: int, ps: int, rn: int):
    key = (NTB, ps, rn)
    if key not in _BUILD_CACHE:
        _BUILD_CACHE[key] = _build(NTB, ps, rn)
    return _BUILD_CACHE[key]


def _pack(Kv, k1v, adj, sm, NTB):
    """Pack one core-branch: all 4 batch rows' adj=1 tokens concatenated,
    token j at partition j//NTB slot j%NTB; x gets [K | k1 | 1 | 0] rows."""
    f16 = np.float16
    CBc = 128 * NTB
    xc = np.zeros((CBc, FP), dtype=f16)
    m8 = np.zeros((128, NTB, 8), dtype=f16)
    pos = 0
    for b in range(Kv.shape[0]):
        idx = np.flatnonzero(adj[b])
        k = len(idx)
        xc[pos : pos + k, 0:D] = Kv[b, idx]
        xc[pos : pos + k, D:F] = k1v[b, idx]
        xc[pos : pos + k, F] = 1.0
        sl = sm[b, idx].astype(f16)
        j = np.arange(pos, pos + k)
        m8[j // NTB, j % NTB, b] = sl
        m8[j // NTB, j % NTB, 4 + b] = 1.0 - sl
        pos += k
    return xc, m8


def kernel(**inputs) -> tuple:
    global last_results
    from concourse.bass_utils import run_bass_kernel_spmd

    f32 = np.float32
    f16 = np.float16
    K = np.asarray(inputs["K"], dtype=f32)
    front_k1 = np.asarray(inputs["front_k1"], dtype=f32)
    back_K = np.asarray(inputs["back_K"], dtype=f32)
    back_k2 = np.asarray(inputs["back_k2"], dtype=f32)
    Wfk = np.asarray(inputs["Wfk"], dtype=f32)
    bfk = np.asarray(inputs["bfk"], dtype=f32)
    Wbk = np.asarray(inputs["Wbk"], dtype=f32)
    bbk = np.asarray(inputs["bbk"], dtype=f32)
    Wr0 = np.asarray(inputs["Wr0"], dtype=f32)
    Wr1 = np.asarray(inputs["Wr1"], dtype=f32)
    wf_den = np.asarray(inputs["wf_den"], dtype=f32)
    wb_den = np.asarray(inputs["wb_den"], dtype=f32)
    adj_f = np.asarray(inputs["front_sdj_den"], dtype=np.int32)
    sm_f = np.asarray(inputs["front_s_mask"], dtype=np.int32)
    adj_b = np.asarray(inputs["back_sdj_den"], dtype=np.int32)
    sm_b = np.asarray(inputs["back_s_mask"], dtype=np.int32)
    i = int(np.asarray(inputs["i"]))
    num_utter = int(np.asarray(inputs["num_utter"]))

    # host-folded weights
    v_f = (Wfk.astype(np.float64) @ wf_den[D:].astype(np.float64)).astype(f16)
    v_b = (Wbk.astype(np.float64) @ wb_den[D:].astype(np.float64)).astype(f16)
    A_f = np.vstack([Wfk, bfk[None, :]]).astype(np.float64)
    A_b = np.vstack([Wbk, bbk[None, :]]).astype(np.float64)
    G0_f = (A_f @ Wr0.astype(np.float64)).astype(f16)
    G1_f = (A_f @ Wr1.astype(np.float64)).astype(f16)
    G0_b = (A_b @ Wr0.astype(np.float64)).astype(f16)
    G1_b = (A_b @ Wr1.astype(np.float64)).astype(f16)

    # packed context length (adj=0 tokens contribute exactly 0)
    per_cb_f = adj_f.reshape(NCORES, BL, N).sum(axis=(1, 2))
    per_cb_b = adj_b.reshape(NCORES, BL, N).sum(axis=(1, 2))
    maxcnt = max(int(per_cb_f.max()), int(per_cb_b.max()), 1)
    NTB = min((BL * N) // 128, (maxcnt + 127) // 128)

    ps = int(os.environ.get("KERNEL_PS", "256"))
    rn = int(os.environ.get("KERNEL_RN", "3"))
    nc = _get_nc(NTB, ps, rn)

    in_maps = []
    for c in range(NCORES):
        sl = slice(c * BL, (c + 1) * BL)
        x_f, m8_f = _pack(K[sl], front_k1[sl], adj_f[sl], sm_f[sl], NTB)
        x_b, m8_b = _pack(back_K[sl], back_k2[sl], adj_b[sl], sm_b[sl], NTB)
        in_maps.append(
            {
                "x_f": x_f,
                "x_b": x_b,
                "m8_f": m8_f,
                "m8_b": m8_b,
                "v_f": v_f,
                "v_b": v_b,
                "G0_f": G0_f,
                "G1_f": G1_f,
                "G0_b": G0_b,
                "G1_b": G1_b,
            }
        )

    trace = os.environ.get("KERNEL_TRACE", "0") == "1"
    res = run_bass_kernel_spmd(nc, in_maps, core_ids=list(range(NCORES)), trace=trace)
    last_results = res

    front = np.concatenate([r["out_f"] for r in res.results], axis=0)
    back = np.concatenate([r["out_b"] for r in res.results], axis=0)
    if i == 0:
        front = np.zeros((B, D), dtype=f32)
    if i == num_utter - 1:
        back = np.zeros((B, D), dtype=f32)
    return (front, back)
